# revision 10
# baseline (speedup 1.0000x reference)
"""Trainium2 Bass kernel v2 for nn_Attention_27977416966176.

Computation (per example b):
    hm[b]   = sum_l dec[l, b, :]                 # [H]  (x0.25 via exp scale)
    s[b]    = 0.25 * enc[b] @ hm[b]              # [S]
    w[b]    = softmax(s[b])
    out[b]  = enc[b].T @ w[b]                    # [H]

Key cost-model facts driving the design (measured in CoreSim):
  - A DMA costs ~0.386 ns per free-byte-per-partition on its issuing
    engine's queue, independent of partition count; different engines'
    queues overlap fully. enc streams on TWO queues (SP + Pool), and all
    small tensors are shaped to spread over many partitions.
  - DVE ops with an accumulator never get the 16-bit speedup: a fused
    product+score op costs 1127 ns/tile regardless of dtype (route B).
  - gpsimd (Pool) DMAs convert dtypes in flight, priced at OUTPUT bytes:
    an f16 cast-load of a granule costs 3.16 us, half the f32 price.
    Route C rides this: Pool cast-DMA (f16 enc) + DVE tensor_tensor f16
    product at 2x (594 ns/tile) + ACT identity-with-accum score reduce
    (1225 ns/tile). No explicit cast op exists anywhere. The B/C granule
    mix balances DVE against ACT, and a strict 3/2 alternation of
    per-example C-counts keeps the pipeline's binding engine constant.
  - PE matmul cost = out-free-size x cycles/row (indep. of k): the fp16
    weighted sum costs ~0.21-0.43 us per [1,512] accumulation step.

Both routes produce prod16 = fp16(enc * 64*hm) which feeds the PE weighted
sum; the final result is unscaled by 1/(64*hm) (relative rounding divides
back out exactly, so fp16 precision of enc is preserved; x64 keeps tiny
|hm| products out of the subnormal range). Scores come for free: route B's
STT accumulates f32 pre-cast products; route C's ACT reduce accumulates
the f16 products.

Softmax uses the baseline's constant exp shift (seed-0 scores lie in
[-83, 85]; exp(s - 40) in f32 neither overflows nor loses weights), and
weights are normalized before fp16 (w <= 1 is fp16-safe).

attn rows accumulate in PSUM [1, 512] banks. Examples 0..6 are staged via
a DRAM scratch round-trip into [8, H] rows, unscaled by one batched STT
off the critical path; example 7's unscale is fused into its PSUM
evacuation (STT with a pre-staged partition-0 reciprocal row) so the tail
is one evacuation + one row DMA.
"""

import sys

import numpy as np

try:
    import concourse.bass as bass
except ImportError:  # fall back to the in-container checkout
    sys.path.insert(0, "/opt/trn_rl_repo")
    import concourse.bass as bass

import concourse.bacc as bacc
import concourse.tile as tile
from concourse import mybir
from concourse.bass_utils import run_bass_kernel_spmd

B, S, H, L = 64, 2048, 1024, 4
NCORES = 8
BPC = B // NCORES  # examples per core
P = 128            # SBUF partitions
T = S // P         # s-tiles per example
SHIFT = 40.0       # constant softmax exp shift (see module docstring)
SC = 64.0          # hm scale folded into the fp16 product

F32 = mybir.dt.float32
F16 = mybir.dt.float16
TG = 4             # s-tiles per DMA granule
NG = T // TG       # granules per example

# Granule routes. C granules are cast-DMA'd to f16 by Pool (gpsimd DMAs
# convert dtypes, priced at output bytes: 3.16 us/granule), multiplied by
# hm16 on DVE (f16 2x), and score-reduced on ACT; B granules stream f32
# on SP into the fused DVE op. 20 C granules balance ACT against DVE,
# and the 3/2 alternation minimizes pipeline bubbles (swept).
_CC = [3, 2, 3, 2, 3, 2, 3, 2]
_POS = {2: (0, 2), 3: (0, 1, 3)}
C_GRANULES = {(b, g) for b in range(BPC) for g in _POS[_CC[b]]}


def _ind4() -> np.ndarray:
    # [32, 128] f32: column m (< BPC) selects the 4 layer rows of example m
    ind = np.zeros((L * BPC, P), dtype=np.float32)
    for b in range(BPC):
        ind[b * L : (b + 1) * L, b] = 1.0
    return ind


def _ind16() -> np.ndarray:
    # [8, 8*128] f16: block b is an indicator (row b ones) used as K=8
    # matmul lhsT to broadcast hm16_b row b to 128 partitions
    ind = np.zeros((BPC, BPC * P), dtype=np.float16)
    for b in range(BPC):
        ind[b, b * P : (b + 1) * P] = 1.0
    return ind


def build_program() -> bass.Bass:
    nc = bacc.Bacc("TRN2", target_bir_lowering=False, debug=False)

    enc_d = nc.dram_tensor("enc", [BPC, S, H], F32, kind="ExternalInput")
    dec_d = nc.dram_tensor("dec", [BPC, L, H], F32, kind="ExternalInput")
    ind4_d = nc.dram_tensor("ind4", [L * BPC, P], F32, kind="ExternalInput")
    ind16_d = nc.dram_tensor("ind16", [BPC, BPC * P], F16, kind="ExternalInput")
    out_d = nc.dram_tensor("out", [BPC, H], F32, kind="ExternalOutput")
    # staging scratch for the psum-row -> [8, H] round trip (rows 0..6)
    scratch_d = nc.dram_tensor("scratch", [BPC, H], F32, kind="ExternalOutput")

    # enc[b] rows s = t*128 + p, viewed [b, p, t, h] so any contiguous
    # t-range slices into one (p, t, h)-ordered DMA
    enc_t = enc_d.ap().rearrange("b (t p) h -> b p t h", p=P)

    with tile.TileContext(nc) as tc:
        with (
            tc.tile_pool(name="singles", bufs=1) as singles,
            tc.tile_pool(name="ps_hm", bufs=2, space="PSUM") as ps_hm,
            tc.tile_pool(name="ps_attn", bufs=2, space="PSUM") as ps_attn,
            tc.tile_pool(name="ps_den", bufs=1, space="PSUM") as ps_den,
        ):
            # ---- constants ----
            ones128 = singles.tile([P, P], F32)  # K=128 cross-partition sum
            nc.vector.memset(ones128[:], 1.0)
            neg_shift = singles.tile([P, 1], F32)
            nc.vector.memset(neg_shift[:], -SHIFT)

            # aux loads on SP (it has queue slack; ACT is the peak engine)
            ind4 = singles.tile([L * BPC, P], F32)
            nc.sync.dma_start(out=ind4[:], in_=ind4_d.ap())
            ind16 = singles.tile([BPC, BPC * P], F16)
            nc.sync.dma_start(out=ind16[:], in_=ind16_d.ap())
            dec32 = singles.tile([L * BPC, H], F32)  # [ (b,l), h ]
            nc.sync.dma_start(
                out=dec32[:], in_=dec_d.ap().rearrange("b l h -> (b l) h")
            )

            # hm_b[b, :] = sum_l dec[b, l, :] via one K=32 indicator matmul
            # (m=128 columns; only the first 8 are used, same psum shape as
            # the broadcast matmuls so the pool shares slots)
            hm_ps = ps_hm.tile([P, H], F32, tag="hmps")
            for j in range(2):
                nc.tensor.matmul(
                    out=hm_ps[:, j * 512 : (j + 1) * 512],
                    lhsT=ind4[:],
                    rhs=dec32[:, j * 512 : (j + 1) * 512],
                    start=True, stop=True,
                )
            hm_b = singles.tile([BPC, H], F32)
            nc.vector.tensor_copy(hm_b[:], hm_ps[:BPC, :])
            recip_raw = singles.tile([BPC, H], F32)  # 1 / hm_b
            nc.vector.reciprocal(recip_raw[:], hm_b[:])
            hm16_b = singles.tile([BPC, H], F16)     # fp16(64 * hm_b)
            nc.scalar.activation(
                out=hm16_b[:], in_=hm_b[:],
                func=mybir.ActivationFunctionType.Copy, scale=SC,
            )
            # examples 6/7's reciprocal rows, needed at partition 0 for the
            # fused unscale-evacuation: computed from row 0 of their hm
            # broadcast psums (replicated hm at partition 0; engine ops are
            # dependency-tracked, so nothing can reorder past them). NOTE
            # these come from hm16_b so they carry its fp16 rounding; the
            # product shares it, so the division still cancels exactly.
            recip67 = singles.tile([1, 2, H], F32)

            # hm16s[:, b, :] = fp16(64*hm_b[b]) on all 128 partitions
            hm16s = singles.tile([P, BPC, H], F16)
            for b in range(BPC):
                hmb_ps = ps_hm.tile([P, H], F32, tag="hmps")
                for j in range(2):
                    nc.tensor.matmul(
                        out=hmb_ps[:, j * 512 : (j + 1) * 512],
                        lhsT=ind16[:, b * P : (b + 1) * P],
                        rhs=hm16_b[:, j * 512 : (j + 1) * 512],
                        start=True, stop=True,
                    )
                if b in (3,):
                    nc.vector.tensor_copy(hm16s[:, b, :], hmb_ps[:])
                else:
                    nc.scalar.copy(hm16s[:, b, :], hmb_ps[:])
                if b >= BPC - 2:
                    # psum rows hold 64*hm16(b); fold the 1/64 here so the
                    # fused evac keeps its 1/SC scalar
                    nc.vector.reciprocal(
                        recip67[:, b - (BPC - 2), :], hmb_ps[0:1, :]
                    )

            # ---- per-example pipeline ----
            with (
                tc.tile_pool(name="encp", bufs=3) as encp,
                tc.tile_pool(name="encp16", bufs=4) as encp16,
                tc.tile_pool(name="prodp", bufs=2) as prodp,
                tc.tile_pool(name="small", bufs=2) as small,
            ):
                sink = singles.tile([P, H], F16)  # ACT-reduce sink
                attn_all = singles.tile([BPC, H], F32)
                nc.vector.memset(attn_all[:], 0.0)

                for b in range(BPC):
                    prod16 = prodp.tile([P, T, H], F16)
                    scores = small.tile([P, T], F32)

                    # taper: last example ends on a 1-tile granule so the
                    # final serial score chain is short
                    if b == BPC - 1:
                        plan = [TG] * (NG - 1) + [TG - 1, 1]
                    else:
                        plan = [TG] * NG

                    t0 = 0
                    for g, gsz in enumerate(plan):
                        if (b, g) in C_GRANULES:
                            # route C: Pool cast-DMA delivers f16 directly
                            # (gpsimd DMAs convert dtypes at output-byte
                            # cost); DVE f16 product (2x); ACT reduce
                            enc16 = encp16.tile([P, TG, H], F16, tag="encg16")
                            nc.gpsimd.dma_start(
                                out=enc16[:, :gsz, :],
                                in_=enc_t[b, :, t0 : t0 + gsz, :],
                            )
                            for t in range(gsz):
                                nc.vector.tensor_tensor(
                                    out=prod16[:, t0 + t, :],
                                    in0=enc16[:, t, :],
                                    in1=hm16s[:, b, :],
                                    op=mybir.AluOpType.mult,
                                )
                            for t in range(gsz):
                                nc.scalar.activation(
                                    out=sink[:],
                                    in_=prod16[:, t0 + t, :],
                                    func=mybir.ActivationFunctionType.Copy,
                                    scale=1.0,
                                    accum_out=scores[:, t0 + t : t0 + t + 1],
                                )
                        else:
                            # route B: one fused DVE op -> f16 product plus
                            # f32 pre-cast score accumulation
                            encg = encp.tile([P, TG, H], F32, tag="encg")
                            nc.sync.dma_start(
                                out=encg[:, :gsz, :],
                                in_=enc_t[b, :, t0 : t0 + gsz, :],
                            )
                            for t in range(gsz):
                                nc.vector.scalar_tensor_tensor(
                                    out=prod16[:, t0 + t, :],
                                    in0=encg[:, t, :],
                                    scalar=1.0,
                                    in1=hm16s[:, b, :],
                                    op0=mybir.AluOpType.mult,
                                    op1=mybir.AluOpType.mult,
                                    accum_out=scores[:, t0 + t : t0 + t + 1],
                                )
                        t0 += gsz

                    # softmax: wexp = exp(scores/256 - 40); accum row sums.
                    # scores hold 256*(mean-layer scores): 64 from hm16s'
                    # scale and 4 from summing instead of averaging layers.
                    wexp = small.tile([P, T], F32)
                    row_sums = small.tile([P, 1], F32)
                    nc.scalar.activation(
                        out=wexp[:], in_=scores[:],
                        func=mybir.ActivationFunctionType.Exp,
                        bias=neg_shift[:], scale=1.0 / 256.0,
                        accum_out=row_sums[:],
                    )

                    # denominator replicated across partitions (K=128
                    # ones-matmul), reciprocal, normalized fp16 weights
                    den_ps = ps_den.tile([P, 1], F32)
                    nc.tensor.matmul(
                        out=den_ps[:], lhsT=ones128[:], rhs=row_sums[:],
                        start=True, stop=True,
                    )
                    recip_rep = small.tile([P, 1], F32)
                    nc.vector.reciprocal(recip_rep[:], den_ps[:])
                    w16 = small.tile([P, T], F16)
                    nc.vector.tensor_scalar_mul(w16[:], wexp[:], recip_rep[:])

                    # weighted sum on PE: attn_j[1, 512] += w16.T @ prod16
                    attn_ps_j = []
                    for j in range(2):
                        attn_ps = ps_attn.tile([1, 512], F32)
                        for t in range(T):
                            nc.tensor.matmul(
                                out=attn_ps[:],
                                lhsT=w16[:, t : t + 1],
                                rhs=prod16[:, t, j * 512 : (j + 1) * 512],
                                start=(t == 0),
                                stop=(t == T - 1),
                            )
                        attn_ps_j.append(attn_ps)

                    if b >= BPC - 2:
                        # fused unscale-evacuation for the two examples at
                        # the pipeline tail: out_b = psum/(64*hm_b)
                        outb = small.tile([1, H], F32, tag="attn_sb")
                        for j in range(2):
                            # recip67 = 1/(64*hm) already includes the 1/SC
                            nc.vector.scalar_tensor_tensor(
                                out=outb[:, j * 512 : (j + 1) * 512],
                                in0=attn_ps_j[j][:],
                                scalar=1.0,
                                in1=recip67[:, b - (BPC - 2),
                                            j * 512 : (j + 1) * 512],
                                op0=mybir.AluOpType.mult,
                                op1=mybir.AluOpType.mult,
                            )
                        nc.sync.dma_start(
                            out=out_d.ap()[b : b + 1, :], in_=outb[:]
                        )
                    else:
                        attn_sb = small.tile([1, H], F32, tag="attn_sb")
                        for j in range(2):
                            if b % 2 == 0:
                                nc.scalar.copy(
                                    attn_sb[:, j * 512 : (j + 1) * 512],
                                    attn_ps_j[j][:],
                                )
                            else:
                                nc.vector.tensor_copy(
                                    attn_sb[:, j * 512 : (j + 1) * 512],
                                    attn_ps_j[j][:],
                                )
                        nc.sync.dma_start(
                            out=scratch_d.ap()[b : b + 1, :], in_=attn_sb[:]
                        )

                    if b == BPC - 3:
                        # batched finalize of rows 0..5 (off critical path;
                        # same-queue SP ordering serializes scratch writes
                        # before this gather): out = attn/(64*hm)
                        nc.sync.dma_start(
                            out=attn_all[: BPC - 2, :],
                            in_=scratch_d.ap()[: BPC - 2, :],
                        )
                        nc.vector.scalar_tensor_tensor(
                            out=attn_all[:], in0=attn_all[:],
                            scalar=1.0 / SC, in1=recip_raw[:],
                            op0=mybir.AluOpType.mult,
                            op1=mybir.AluOpType.mult,
                        )
                        nc.sync.dma_start(
                            out=out_d.ap()[: BPC - 2, :],
                            in_=attn_all[: BPC - 2, :],
                        )

    nc.finalize()
    return nc


def run(encoder_outputs: np.ndarray, decoder_hidden: np.ndarray, **spmd_kwargs):
    """Run the kernel; returns (output [B, 1, H], BassKernelResults)."""
    assert encoder_outputs.shape == (B, S, H)
    assert decoder_hidden.shape == (L, B, H)
    enc = np.ascontiguousarray(encoder_outputs, dtype=np.float32)
    # [L, B, H] -> [B, L, H] so each core's shard is a clean slice
    dec = np.ascontiguousarray(
        np.transpose(decoder_hidden, (1, 0, 2)), dtype=np.float32
    )

    nc = build_program()

    ind4 = _ind4()
    ind16 = _ind16()
    in_maps = []
    for c in range(NCORES):
        lo, hi = c * BPC, (c + 1) * BPC
        in_maps.append(
            {
                "enc": np.ascontiguousarray(enc[lo:hi]),
                "dec": np.ascontiguousarray(dec[lo:hi]),
                "ind4": ind4,
                "ind16": ind16,
            }
        )

    res = run_bass_kernel_spmd(
        nc, in_maps, core_ids=list(range(NCORES)), **spmd_kwargs
    )
    out = np.concatenate([res.results[c]["out"] for c in range(NCORES)], axis=0)
    return out.reshape(B, 1, H), res


def kernel(encoder_outputs: np.ndarray, decoder_hidden: np.ndarray) -> np.ndarray:
    out, _ = run(encoder_outputs, decoder_hidden)
    return out


# revision 11
# speedup vs baseline: 1.0097x; 1.0097x over previous
"""Trainium2 Bass kernel v2 for nn_Attention_27977416966176.

Computation (per example b):
    hm[b]   = sum_l dec[l, b, :]                 # [H]  (x0.25 via exp scale)
    s[b]    = 0.25 * enc[b] @ hm[b]              # [S]
    w[b]    = softmax(s[b])
    out[b]  = enc[b].T @ w[b]                    # [H]

Key cost-model facts driving the design (measured in CoreSim):
  - A DMA costs ~0.386 ns per free-byte-per-partition on its issuing
    engine's queue, independent of partition count; different engines'
    queues overlap fully. enc streams on TWO queues (SP + Pool), and all
    small tensors are shaped to spread over many partitions.
  - DVE ops with an accumulator never get the 16-bit speedup: a fused
    product+score op costs 1127 ns/tile regardless of dtype (route B).
  - gpsimd (Pool) DMAs convert dtypes in flight, priced at OUTPUT bytes:
    an f16 cast-load of a granule costs 3.16 us, half the f32 price.
    Route C rides this: Pool cast-DMA (f16 enc) + DVE tensor_tensor f16
    product at 2x (594 ns/tile) + ACT identity-with-accum score reduce
    (1225 ns/tile). No explicit cast op exists anywhere. The B/C granule
    mix balances DVE against ACT, and a strict 3/2 alternation of
    per-example C-counts keeps the pipeline's binding engine constant.
  - PE matmul cost = out-free-size x cycles/row (indep. of k): the fp16
    weighted sum costs ~0.21-0.43 us per [1,512] accumulation step.

Both routes produce prod16 = fp16(enc * 64*hm) which feeds the PE weighted
sum; the final result is unscaled by 1/(64*hm) (relative rounding divides
back out exactly, so fp16 precision of enc is preserved; x64 keeps tiny
|hm| products out of the subnormal range). Scores come for free: route B's
STT accumulates f32 pre-cast products; route C's ACT reduce accumulates
the f16 products.

Softmax uses the baseline's constant exp shift (seed-0 scores lie in
[-83, 85]; exp(s - 40) in f32 neither overflows nor loses weights), and
weights are normalized before fp16 (w <= 1 is fp16-safe).

attn rows accumulate in PSUM [1, 512] banks. Examples 0..6 are staged via
a DRAM scratch round-trip into [8, H] rows, unscaled by one batched STT
off the critical path; example 7's unscale is fused into its PSUM
evacuation (STT with a pre-staged partition-0 reciprocal row) so the tail
is one evacuation + one row DMA.
"""

import sys

import numpy as np

try:
    import concourse.bass as bass
except ImportError:  # fall back to the in-container checkout
    sys.path.insert(0, "/opt/trn_rl_repo")
    import concourse.bass as bass

import concourse.bacc as bacc
import concourse.tile as tile
from concourse import mybir
from concourse.bass_utils import run_bass_kernel_spmd

B, S, H, L = 64, 2048, 1024, 4
NCORES = 8
BPC = B // NCORES  # examples per core
P = 128            # SBUF partitions
T = S // P         # s-tiles per example
SHIFT = 40.0       # constant softmax exp shift (see module docstring)
SC = 64.0          # hm scale folded into the fp16 product

F32 = mybir.dt.float32
F16 = mybir.dt.float16
TG = 4             # s-tiles per DMA granule
NG = T // TG       # granules per example

# Granule routes. C granules are cast-DMA'd to f16 by Pool (gpsimd DMAs
# convert dtypes, priced at output bytes: 3.16 us/granule), multiplied by
# hm16 on DVE (f16 2x), and score-reduced on ACT; B granules stream f32
# on SP into the fused DVE op. 20 C granules balance ACT against DVE,
# and the 3/2 alternation minimizes pipeline bubbles (swept).
_CC = [3, 2, 3, 2, 3, 2, 3, 2]
_POS = {2: (0, 2), 3: (0, 1, 3)}
C_GRANULES = {(b, g) for b in range(BPC) for g in _POS[_CC[b]]}


def _ind4() -> np.ndarray:
    # [32, 128] f32: column m (< BPC) selects the 4 layer rows of example m
    ind = np.zeros((L * BPC, P), dtype=np.float32)
    for b in range(BPC):
        ind[b * L : (b + 1) * L, b] = 1.0
    return ind


def _ind16() -> np.ndarray:
    # [8, 8*128] f16: block b is an indicator (row b ones) used as K=8
    # matmul lhsT to broadcast hm16_b row b to 128 partitions
    ind = np.zeros((BPC, BPC * P), dtype=np.float16)
    for b in range(BPC):
        ind[b, b * P : (b + 1) * P] = 1.0
    return ind


def build_program() -> bass.Bass:
    nc = bacc.Bacc("TRN2", target_bir_lowering=False, debug=False)

    enc_d = nc.dram_tensor("enc", [BPC, S, H], F32, kind="ExternalInput")
    dec_d = nc.dram_tensor("dec", [BPC, L, H], F32, kind="ExternalInput")
    ind4_d = nc.dram_tensor("ind4", [L * BPC, P], F32, kind="ExternalInput")
    ind16_d = nc.dram_tensor("ind16", [BPC, BPC * P], F16, kind="ExternalInput")
    out_d = nc.dram_tensor("out", [BPC, H], F32, kind="ExternalOutput")
    # staging scratch for the psum-row -> [8, H] round trip (rows 0..6)
    scratch_d = nc.dram_tensor("scratch", [BPC, H], F32, kind="ExternalOutput")

    # enc[b] rows s = t*128 + p, viewed [b, p, t, h] so any contiguous
    # t-range slices into one (p, t, h)-ordered DMA
    enc_t = enc_d.ap().rearrange("b (t p) h -> b p t h", p=P)

    with tile.TileContext(nc) as tc:
        with (
            tc.tile_pool(name="singles", bufs=1) as singles,
            tc.tile_pool(name="ps_hm", bufs=2, space="PSUM") as ps_hm,
            tc.tile_pool(name="ps_attn", bufs=2, space="PSUM") as ps_attn,
            tc.tile_pool(name="ps_den", bufs=1, space="PSUM") as ps_den,
        ):
            # ---- constants ----
            ones128 = singles.tile([P, P], F32)  # K=128 cross-partition sum
            nc.vector.memset(ones128[:], 1.0)
            neg_shift = singles.tile([P, 1], F32)
            nc.vector.memset(neg_shift[:], -SHIFT)

            # aux loads on SP (it has queue slack; ACT is the peak engine)
            ind4 = singles.tile([L * BPC, P], F32)
            nc.sync.dma_start(out=ind4[:], in_=ind4_d.ap())
            ind16 = singles.tile([BPC, BPC * P], F16)
            nc.sync.dma_start(out=ind16[:], in_=ind16_d.ap())
            dec32 = singles.tile([L * BPC, H], F32)  # [ (b,l), h ]
            nc.sync.dma_start(
                out=dec32[:], in_=dec_d.ap().rearrange("b l h -> (b l) h")
            )

            # hm_b[b, :] = sum_l dec[b, l, :] via one K=32 indicator matmul
            # (m=128 columns; only the first 8 are used, same psum shape as
            # the broadcast matmuls so the pool shares slots)
            hm_ps = ps_hm.tile([P, H], F32, tag="hmps")
            for j in range(2):
                nc.tensor.matmul(
                    out=hm_ps[:, j * 512 : (j + 1) * 512],
                    lhsT=ind4[:],
                    rhs=dec32[:, j * 512 : (j + 1) * 512],
                    start=True, stop=True,
                )
            hm_b = singles.tile([BPC, H], F32)
            nc.vector.tensor_copy(hm_b[:], hm_ps[:BPC, :])
            recip_raw = singles.tile([BPC, H], F32)  # 1 / hm_b
            nc.vector.reciprocal(recip_raw[:], hm_b[:])
            hm16_b = singles.tile([BPC, H], F16)     # fp16(64 * hm_b)
            nc.scalar.activation(
                out=hm16_b[:], in_=hm_b[:],
                func=mybir.ActivationFunctionType.Copy, scale=SC,
            )
            # examples 6/7's reciprocal rows, needed at partition 0 for the
            # fused unscale-evacuation: computed from row 0 of their hm
            # broadcast psums (replicated hm at partition 0; engine ops are
            # dependency-tracked, so nothing can reorder past them). NOTE
            # these come from hm16_b so they carry its fp16 rounding; the
            # product shares it, so the division still cancels exactly.
            recip67 = singles.tile([1, 2, H], F32)

            # hm16s[:, b, :] = fp16(64*hm_b[b]) on all 128 partitions
            hm16s = singles.tile([P, BPC, H], F16)
            for b in range(BPC):
                hmb_ps = ps_hm.tile([P, H], F32, tag="hmps")
                for j in range(2):
                    nc.tensor.matmul(
                        out=hmb_ps[:, j * 512 : (j + 1) * 512],
                        lhsT=ind16[:, b * P : (b + 1) * P],
                        rhs=hm16_b[:, j * 512 : (j + 1) * 512],
                        start=True, stop=True,
                    )
                if b in (3,):
                    nc.vector.tensor_copy(hm16s[:, b, :], hmb_ps[:])
                else:
                    nc.scalar.copy(hm16s[:, b, :], hmb_ps[:])
                if b >= BPC - 2:
                    # psum rows hold 64*hm16(b); fold the 1/64 here so the
                    # fused evac keeps its 1/SC scalar
                    nc.vector.reciprocal(
                        recip67[:, b - (BPC - 2), :], hmb_ps[0:1, :]
                    )

            # ---- per-example pipeline ----
            with (
                tc.tile_pool(name="encp", bufs=3) as encp,
                tc.tile_pool(name="encp16", bufs=4) as encp16,
                tc.tile_pool(name="prodp", bufs=2) as prodp,
                tc.tile_pool(name="small", bufs=2) as small,
            ):
                sink = singles.tile([P, H], F16)  # ACT-reduce sink
                attn_all = singles.tile([BPC, H], F32)
                nc.vector.memset(attn_all[:], 0.0)

                for b in range(BPC):
                    prod16 = prodp.tile([P, T, H], F16)
                    scores = small.tile([P, T], F32)

                    # taper: last example ends on a 1-tile granule so the
                    # final serial score chain is short
                    if b == BPC - 1:
                        plan = [TG] * (NG - 1) + [TG - 1, 1]
                    else:
                        plan = [TG] * NG

                    t0 = 0
                    for g, gsz in enumerate(plan):
                        if (b, g) in C_GRANULES:
                            # route C: Pool cast-DMA delivers f16 directly
                            # (gpsimd DMAs convert dtypes at output-byte
                            # cost); DVE f16 product (2x); ACT reduce
                            enc16 = encp16.tile([P, TG, H], F16, tag="encg16")
                            nc.gpsimd.dma_start(
                                out=enc16[:, :gsz, :],
                                in_=enc_t[b, :, t0 : t0 + gsz, :],
                            )
                            for t in range(gsz):
                                nc.vector.tensor_tensor(
                                    out=prod16[:, t0 + t, :],
                                    in0=enc16[:, t, :],
                                    in1=hm16s[:, b, :],
                                    op=mybir.AluOpType.mult,
                                )
                            for t in range(gsz):
                                nc.scalar.activation(
                                    out=sink[:],
                                    in_=prod16[:, t0 + t, :],
                                    func=mybir.ActivationFunctionType.Copy,
                                    scale=1.0,
                                    accum_out=scores[:, t0 + t : t0 + t + 1],
                                )
                        else:
                            # route B: one fused DVE op -> f16 product plus
                            # f32 pre-cast score accumulation
                            encg = encp.tile([P, TG, H], F32, tag="encg")
                            nc.sync.dma_start(
                                out=encg[:, :gsz, :],
                                in_=enc_t[b, :, t0 : t0 + gsz, :],
                            )
                            for t in range(gsz):
                                nc.vector.scalar_tensor_tensor(
                                    out=prod16[:, t0 + t, :],
                                    in0=encg[:, t, :],
                                    scalar=1.0,
                                    in1=hm16s[:, b, :],
                                    op0=mybir.AluOpType.mult,
                                    op1=mybir.AluOpType.mult,
                                    accum_out=scores[:, t0 + t : t0 + t + 1],
                                )
                        t0 += gsz

                    # softmax: wexp = exp(scores/256 - 40); accum row sums.
                    # scores hold 256*(mean-layer scores): 64 from hm16s'
                    # scale and 4 from summing instead of averaging layers.
                    wexp = small.tile([P, T], F32)
                    row_sums = small.tile([P, 1], F32)
                    nc.scalar.activation(
                        out=wexp[:], in_=scores[:],
                        func=mybir.ActivationFunctionType.Exp,
                        bias=neg_shift[:], scale=1.0 / 256.0,
                        accum_out=row_sums[:],
                    )

                    # denominator replicated across partitions (K=128
                    # ones-matmul), reciprocal, normalized fp16 weights
                    den_ps = ps_den.tile([P, 1], F32)
                    nc.tensor.matmul(
                        out=den_ps[:], lhsT=ones128[:], rhs=row_sums[:],
                        start=True, stop=True,
                    )
                    recip_rep = small.tile([P, 1], F32)
                    nc.vector.reciprocal(recip_rep[:], den_ps[:])
                    w16 = small.tile([P, T], F16)
                    nc.vector.tensor_scalar_mul(w16[:], wexp[:], recip_rep[:])

                    # weighted sum on PE: attn_j[1, 512] += w16.T @ prod16
                    attn_ps_j = []
                    for j in range(2):
                        attn_ps = ps_attn.tile([1, 512], F32)
                        for t in range(T):
                            nc.tensor.matmul(
                                out=attn_ps[:],
                                lhsT=w16[:, t : t + 1],
                                rhs=prod16[:, t, j * 512 : (j + 1) * 512],
                                start=(t == 0),
                                stop=(t == T - 1),
                            )
                        attn_ps_j.append(attn_ps)

                    if b >= BPC - 2:
                        # fused unscale-evacuation for the two examples at
                        # the pipeline tail: out_b = psum/(64*hm_b)
                        outb = small.tile([1, H], F32, tag="attn_sb")
                        for j in range(2):
                            # recip67 = 1/(64*hm) already includes the 1/SC
                            nc.vector.scalar_tensor_tensor(
                                out=outb[:, j * 512 : (j + 1) * 512],
                                in0=attn_ps_j[j][:],
                                scalar=1.0,
                                in1=recip67[:, b - (BPC - 2),
                                            j * 512 : (j + 1) * 512],
                                op0=mybir.AluOpType.mult,
                                op1=mybir.AluOpType.mult,
                            )
                        nc.sync.dma_start(
                            out=out_d.ap()[b : b + 1, :], in_=outb[:]
                        )
                    else:
                        attn_sb = small.tile([1, H], F32, tag="attn_sb")
                        for j in range(2):
                            if b % 2 == 0 or b == 5:
                                nc.scalar.copy(
                                    attn_sb[:, j * 512 : (j + 1) * 512],
                                    attn_ps_j[j][:],
                                )
                            else:
                                nc.vector.tensor_copy(
                                    attn_sb[:, j * 512 : (j + 1) * 512],
                                    attn_ps_j[j][:],
                                )
                        nc.sync.dma_start(
                            out=scratch_d.ap()[b : b + 1, :], in_=attn_sb[:]
                        )

                    if b == BPC - 3:
                        # batched finalize of rows 0..5 (off critical path;
                        # same-queue SP ordering serializes scratch writes
                        # before this gather): out = attn/(64*hm)
                        nc.sync.dma_start(
                            out=attn_all[: BPC - 2, :],
                            in_=scratch_d.ap()[: BPC - 2, :],
                        )
                        nc.vector.scalar_tensor_tensor(
                            out=attn_all[:], in0=attn_all[:],
                            scalar=1.0 / SC, in1=recip_raw[:],
                            op0=mybir.AluOpType.mult,
                            op1=mybir.AluOpType.mult,
                        )
                        nc.sync.dma_start(
                            out=out_d.ap()[: BPC - 2, :],
                            in_=attn_all[: BPC - 2, :],
                        )

    nc.finalize()
    return nc


def run(encoder_outputs: np.ndarray, decoder_hidden: np.ndarray, **spmd_kwargs):
    """Run the kernel; returns (output [B, 1, H], BassKernelResults)."""
    assert encoder_outputs.shape == (B, S, H)
    assert decoder_hidden.shape == (L, B, H)
    enc = np.ascontiguousarray(encoder_outputs, dtype=np.float32)
    # [L, B, H] -> [B, L, H] so each core's shard is a clean slice
    dec = np.ascontiguousarray(
        np.transpose(decoder_hidden, (1, 0, 2)), dtype=np.float32
    )

    nc = build_program()

    ind4 = _ind4()
    ind16 = _ind16()
    in_maps = []
    for c in range(NCORES):
        lo, hi = c * BPC, (c + 1) * BPC
        in_maps.append(
            {
                "enc": np.ascontiguousarray(enc[lo:hi]),
                "dec": np.ascontiguousarray(dec[lo:hi]),
                "ind4": ind4,
                "ind16": ind16,
            }
        )

    res = run_bass_kernel_spmd(
        nc, in_maps, core_ids=list(range(NCORES)), **spmd_kwargs
    )
    out = np.concatenate([res.results[c]["out"] for c in range(NCORES)], axis=0)
    return out.reshape(B, 1, H), res


def kernel(encoder_outputs: np.ndarray, decoder_hidden: np.ndarray) -> np.ndarray:
    out, _ = run(encoder_outputs, decoder_hidden)
    return out
